# revision 63
# baseline (speedup 1.0000x reference)
# Trainium2 Bass kernel for CausalSelfAttention (B=2, T=2048, C=1024, NH=16)
# with interleaved RoPE, sharded over 8 NeuronCores: each core computes one
# batch's 4 heads (data-parallel on batch x tensor-parallel on heads).
#
# Matmul datapath in bf16 (PE streams bf16 at 1 col/cycle @2.4GHz vs fp32's
# ~2.4 cycles/col); PSUM accumulation stays fp32.
#
# Per-core device algorithm:
#   inputs (host pre-laid-out): xt = x[b].T (C,T) bf16;  wt = Wsel.T (C,768)
#   bf16 where Wsel rows = [q-heads | k-heads | v-heads], q/k head rows
#   permuted to [e0..e15, o0..o15, e16..e31, o16..o31] so the RoPE partner
#   lives 16 partitions away inside a 32-partition quadrant; trig = (2,64,T)
#   bf16 RoPE multiplier patterns [CC, SS] (q versions pre-scaled by 1/8 via
#   the weights).
#   phase 1: qkT m-blocks (128 rows = 2 heads) = wt_m.T @ xt, RoPE applied as
#            qk' = raw*CC + shuffle16(raw)*SS  (DVE stream_shuffle swaps
#            16-row halves per quadrant; the two multiplies and the add run
#            on the otherwise-idle Pool engine); v = xt.T @ wt_v in natural
#            (T, d) layout with a ones-column appended (row-sum trick).
#   phase 2: per (head, 512-wide q-chunk): scoresT tiles (128 k, 512 q) on PE
#            into a 3-deep psum pool; causal masking of diagonal tiles via
#            gpsimd affine_select with fill=-1e30 ON THE PSUM, then a single
#            fused exp per 2-ktile group on ACT (no max-subtraction needed:
#            |scores| < ~4) writing bf16 probs; pv on PE accumulating
#            yT_ext (65, 512) whose row 64 = softmax denominators,
#            PE-transpose back to (q, d), scale by reciprocal, DMA out.
import sys

if "/opt/trn_rl_repo" not in sys.path:
    sys.path.insert(0, "/opt/trn_rl_repo")

import numpy as np
import ml_dtypes

BF16_NP = np.dtype(ml_dtypes.bfloat16)

B, T, C, NH, HD = 2, 2048, 1024, 16, 64
NCORES = 8
NCT = 8        # C tiles of 128
NCH = 4        # T chunks of 512
TCH = 512
NKT = 16       # k tiles of 128

PERM = np.array(
    [2 * i for i in range(16)]
    + [2 * i + 1 for i in range(16)]
    + [2 * i for i in range(16, 32)]
    + [2 * i + 1 for i in range(16, 32)],
    dtype=np.int64,
)
FREQ_OF_ROW = np.array(
    list(range(16)) + list(range(16)) + list(range(16, 32)) + list(range(16, 32)),
    dtype=np.int64,
)
IS_ODD_SLOT = np.array([0] * 16 + [1] * 16 + [0] * 16 + [1] * 16, dtype=np.int64)
SHUF_MASK = list(range(16, 32)) + list(range(16))

_CACHE: dict = {}
LAST_RESULTS = None


def _build_nc(phase2=True, rope=True, do_exp=True, do_mask=True, sps_bufs=2,
              pvps_bufs=2, stage_bufs=2, probs_bufs=4, key=None):
    ck = key or (phase2, rope, do_exp, do_mask, sps_bufs, pvps_bufs, stage_bufs,
                 probs_bufs)
    if ck in _CACHE:
        return _CACHE[ck]
    from concourse import bacc
    import concourse.tile as tile
    import concourse.mybir as mybir
    from concourse.masks import make_identity

    F32 = mybir.dt.float32
    BF16 = mybir.dt.bfloat16
    Exp = mybir.ActivationFunctionType.Exp
    Copy = mybir.ActivationFunctionType.Copy

    nc = bacc.Bacc(
        "TRN2",
        target_bir_lowering=False,
        debug=False,
        enable_asserts=False,
        num_devices=NCORES,
    )
    xt_d = nc.dram_tensor("xt", [NCT, 128, T], BF16, kind="ExternalInput")
    wt_d = nc.dram_tensor("wt", [NCT, 128, 768], BF16, kind="ExternalInput")
    trig_d = nc.dram_tensor("trig", [2, 64, T], BF16, kind="ExternalInput")
    y_d = nc.dram_tensor("y", [4, T, HD], F32, kind="ExternalOutput")

    with tile.TileContext(nc) as tc:
        with (
            tc.tile_pool(name="const", bufs=1) as constp,
            tc.tile_pool(name="p2sb", bufs=2) as p2sb,
            tc.tile_pool(name="rope", bufs=2) as ropep,
            tc.tile_pool(name="xw", bufs=1) as xwp,
            tc.tile_pool(name="stage", bufs=stage_bufs, space="PSUM") as stagep,
            tc.tile_pool(name="sps", bufs=sps_bufs, space="PSUM") as sps,
            tc.tile_pool(name="pvps", bufs=pvps_bufs, space="PSUM") as pvps,
        ):
            # ---- constants / destination tiles ----
            trig_t = [constp.tile([128, T], BF16, tag=f"trig{i}", name=f"trig{i}")
                      for i in range(2)]
            ident = constp.tile([128, 128], BF16, tag="ident", name="ident")
            make_identity(nc, ident[:])
            qkT = [
                [constp.tile([128, TCH], BF16, tag=f"qk{m}_{j}", name=f"qk{m}_{j}")
                 for j in range(NCH)]
                for m in range(4)
            ]
            vt = [constp.tile([128, 4, 65], BF16, tag=f"v{kt}", name=f"v{kt}")
                  for kt in range(NKT)]
            for kt in range(NKT):
                nc.vector.memset(vt[kt][:, :, 64:65], 1.0)

            # ---- input DMAs: w + x chunk 0 first, trig, then x chunks 1-3.
            # Spread across two engine DMA queues (sync + gpsimd) so the
            # transfers overlap; a single queue caps at ~358GB/s while HBM
            # sustains ~716. trig goes on scalar's queue.
            dmaq = [nc.sync, nc.gpsimd]
            x_t, w_t = [], []
            for ct in range(NCT):
                wtile = xwp.tile([128, 768], BF16, tag=f"w{ct}", name=f"w{ct}")
                dmaq[ct % 2].dma_start(out=wtile, in_=wt_d[ct])
                w_t.append(wtile)
                xtile = xwp.tile([128, NCH, TCH], BF16, tag=f"x{ct}", name=f"x{ct}")
                dmaq[(ct + 1) % 2].dma_start(out=xtile[:, 0, :], in_=xt_d[ct, :, 0:TCH])
                x_t.append(xtile)
            for i in range(2):
                nc.scalar.dma_start(out=trig_t[i][0:64, :], in_=trig_d[i])
                nc.scalar.dma_start(out=trig_t[i][64:128, :], in_=trig_d[i])
            for j in range(1, NCH):
                for ct in range(NCT):
                    dmaq[ct % 2].dma_start(
                        out=x_t[ct][:, j, :], in_=xt_d[ct, :, TCH * j: TCH * (j + 1)]
                    )

            # ---- per-group matmul + drain helpers ----
            def qk_mm(ps, m, j, u):
                nc.tensor.matmul(
                    ps,
                    w_t[u][:, 128 * m: 128 * (m + 1)],
                    x_t[u][:, j, :],
                    start=(u == 0),
                    stop=(u == NCT - 1),
                )

            def qk_drain(ps, m, j):
                if rope:
                    # ACT drains psum->bf16 (it has slack; psum reads are
                    # slow on DVE), then the whole RoPE chain runs on DVE in
                    # pure bf16 (2x 16-bit mode, SBUF-only operands)
                    raw = ropep.tile([128, TCH], BF16, tag="raw", name="raw")
                    nc.scalar.activation(raw, ps, Copy)
                    shuf = ropep.tile([128, TCH], BF16, tag="shuf", name="shuf")
                    nc.vector.stream_shuffle(out=shuf, in_=raw, mask=SHUF_MASK)
                    t1 = ropep.tile([128, TCH], BF16, tag="t1", name="t1")
                    nc.vector.tensor_mul(
                        t1, raw, trig_t[0][:, TCH * j: TCH * (j + 1)]
                    )
                    nc.vector.tensor_mul(
                        shuf, shuf, trig_t[1][:, TCH * j: TCH * (j + 1)]
                    )
                    nc.vector.tensor_add(qkT[m][j], t1, shuf)
                else:
                    nc.vector.tensor_copy(out=qkT[m][j], in_=ps)

            def v_mm(ps, kt, u):
                nc.tensor.matmul(
                    ps,
                    x_t[u][:, kt // 4, 128 * (kt % 4): 128 * (kt % 4) + 128],
                    w_t[u][:, 512:768],
                    start=(u == 0),
                    stop=(u == NCT - 1),
                )

            def v_drain(ps, kt):
                nc.vector.tensor_copy(
                    out=vt[kt][:, :, 0:64],
                    in_=ps.rearrange("p (s d) -> p s d", s=4),
                )

            def stage_wave_quanta(groups):
                # chop each ping-pong pair into 8 u-step quanta (2 matmuls
                # each) + 1 drain quantum, so the scheduler can interleave
                # them into the PE's exp-wait slots inside attention chunks
                quanta = []
                for i in range(0, len(groups), 2):
                    pair = groups[i: i + 2]
                    state = {}

                    def alloc(pair=pair, state=state):
                        if "pss" not in state:
                            state["pss"] = [
                                stagep.tile(
                                    [128, TCH] if kind == "qk" else [128, 256],
                                    F32, tag="st", name="stg",
                                )
                                for kind, a, b in pair
                            ]
                        return state["pss"]

                    for u in range(NCT):
                        def qmm(u=u, pair=pair, alloc=alloc):
                            for (kind, a, b), ps in zip(pair, alloc()):
                                if kind == "qk":
                                    qk_mm(ps, a, b, u)
                                else:
                                    v_mm(ps, a, u)
                        quanta.append(qmm)

                    def qdr(pair=pair, alloc=alloc):
                        for (kind, a, b), ps in zip(pair, alloc()):
                            if kind == "qk":
                                qk_drain(ps, a, b)
                            else:
                                v_drain(ps, a)
                    quanta.append(qdr)
                return quanta

            def stage_waves(groups):
                for q in stage_wave_quanta(groups):
                    q()

            filler = []

            def emit_filler(n):
                for _ in range(n):
                    if filler:
                        filler.pop(0)()

            # ---- attention machinery ----
            pending = [None]

            def finalize(h, j, psum_y):
                yt_sb = p2sb.tile([65, TCH], BF16, tag="yt", name="yt_sb")
                nc.vector.tensor_copy(out=yt_sb, in_=psum_y)
                psum_t = pvps.tile([128, 4, 66], BF16, tag="pv", name="ps_t")
                for s in range(4):
                    nc.tensor.transpose(
                        psum_t[:, s, 0:65],
                        yt_sb[:, 128 * s: 128 * (s + 1)],
                        ident[0:65, 0:65],
                    )
                rec = p2sb.tile([128, 4], F32, tag="rec", name="rec")
                nc.vector.reciprocal(out=rec, in_=psum_t[:, :, 64])
                y_sb = p2sb.tile([128, 4, HD], F32, tag="ysb", name="y_sb")
                for s in range(4):
                    nc.vector.tensor_scalar_mul(
                        out=y_sb[:, s, :],
                        in0=psum_t[:, s, 0:HD],
                        scalar1=rec[:, s: s + 1],
                    )
                nc.sync.dma_start(
                    out=y_d[h, TCH * j: TCH * (j + 1), :].rearrange(
                        "(s p) d -> p s d", p=128
                    ),
                    in_=y_sb,
                )

            pre_emitted = {}

            def make_emitter(h, j):
                qrow = 64 * (h % 2)
                qm, km = h // 2, 2 + h // 2
                qslice = qkT[qm][j][qrow: qrow + 64, :]
                stiles = {}

                def emit_scores(g):
                    ps = sps.tile([128, 2, TCH], F32, tag="s", name="ps_s")
                    for u in range(2):
                        ki = 2 * g + u
                        delta = max(0, 128 * (ki - 4 * j))
                        kslice = qkT[km][ki // 4][
                            qrow: qrow + 64,
                            128 * (ki % 4): 128 * (ki % 4 + 1),
                        ]
                        nc.tensor.matmul(
                            ps[:, u, delta:TCH], kslice, qslice[:, delta:TCH]
                        )
                    stiles[g] = ps

                return emit_scores, stiles

            def attn_chunk(h, j, nxt=None):
                nk = 4 * j + 4
                ng = nk // 2
                if (h, j) in pre_emitted:
                    emit_scores, stiles = pre_emitted.pop((h, j))
                else:
                    emit_scores, stiles = make_emitter(h, j)
                    emit_scores(0)
                if ng > 1:
                    emit_scores(1)
                if pending[0] is not None:
                    pending[0]()
                    pending[0] = None
                psum_y = pvps.tile([65, TCH], F32, tag="pv", name="pv")
                for g in range(ng):
                    psum_s = stiles.pop(g)
                    deltas = [128 * (2 * g + u - 4 * j) for u in range(2)]
                    dmin = max(0, deltas[0])
                    probs = p2sb.tile(
                        [128, 2, TCH], BF16, tag="probs", name="probs",
                        bufs=probs_bufs,
                    )
                    nc.scalar.activation(
                        probs[:, :, dmin:TCH],
                        psum_s[:, :, dmin:TCH],
                        Exp if do_exp else Copy,
                    )
                    if do_mask and deltas[1] > -128:
                        # causal mask of the diagonal 128-blocks (upper
                        # triangle of block -> 0), on the bf16 probs
                        for u in range(2):
                            d = max(0, deltas[u])
                            nc.gpsimd.affine_select(
                                out=probs[:, u, d:d + 128],
                                in_=probs[:, u, d:d + 128],
                                pattern=[[1, 128]],
                                compare_op=mybir.AluOpType.is_ge,
                                fill=0.0,
                                base=0,
                                channel_multiplier=-1,
                            )
                    if g + 2 < ng:
                        emit_scores(g + 2)
                    elif g == ng - 1 and nxt is not None:
                        nem, nst = make_emitter(*nxt)
                        nem(0)
                        pre_emitted[nxt] = (nem, nst)
                    for u in range(2):
                        ki = 2 * g + u
                        d = max(0, 128 * (ki - 4 * j))
                        nc.tensor.matmul(
                            psum_y[:, d:TCH],
                            vt[ki][:, h, :],
                            probs[:, u, d:TCH],
                            start=(ki == 0),
                            stop=(ki == nk - 1),
                        )

                def fin(h=h, j=j, psum_y=psum_y):
                    finalize(h, j, psum_y)

                pending[0] = fin

            # ---- chunk-streamed schedule ----
            # stage j+1's phase-1 waves are chopped into quanta and fed into
            # the exp-wait slot of every group of stage j's attention chunks
            # (emit_filler in attn_chunk), so the PE always has independent
            # work while ACT runs; leftovers flush at the stage boundary.
            # Pre-emit crosses the stage boundary (the needed qkT drains
            # land many quanta ahead).
            chunk_seq = [(h, j) for j in range(NCH) for h in range(4)]

            def stage_groups(j):
                # qk/v mixed per pair: spreads the heavy RoPE drains (DVE)
                # across all four pairs instead of bunching them up front
                qks = [("qk", 0, j), ("qk", 2, j), ("qk", 1, j), ("qk", 3, j)]
                vs = [("v", kt, 0) for kt in range(4 * j, 4 * j + 4)]
                out = []
                for a, b in zip(qks, vs):
                    out += [a, b]
                return out

            stage_waves(stage_groups(0))
            if phase2:
                for j in range(NCH):
                    nxt_groups = stage_groups(j + 1) if j + 1 < NCH else None
                    for h in range(4):
                        pos = chunk_seq.index((h, j))
                        nxt = chunk_seq[pos + 1] if pos + 1 < len(chunk_seq) else None
                        attn_chunk(h, j, nxt=nxt)
                        if nxt_groups is not None:
                            stage_waves(nxt_groups[2 * h: 2 * h + 2])
            else:
                for j in range(1, NCH):
                    stage_waves(stage_groups(j))
            if phase2 and pending[0] is not None:
                pending[0]()
                pending[0] = None

    nc.compile()
    _CACHE[ck] = nc
    return nc


def _host_prep(x, w_attn, freqs_cos, freqs_sin):
    x = np.asarray(x, dtype=np.float32)
    w = np.asarray(w_attn, dtype=np.float32)
    fc = np.asarray(freqs_cos, dtype=np.float32)
    fs = np.asarray(freqs_sin, dtype=np.float32)

    cosT, sinT = fc.T, fs.T                      # (32, T)
    CCp = cosT[FREQ_OF_ROW]                       # (64, T)
    SSp = sinT[FREQ_OF_ROW] * np.where(IS_ODD_SLOT == 1, 1.0, -1.0)[:, None].astype(
        np.float32
    )
    trig = np.ascontiguousarray(np.stack([CCp, SSp])).astype(BF16_NP)
    qscale = np.float32(1.0 / np.sqrt(HD))

    in_maps = []
    for c in range(NCORES):
        b = c // 4
        heads = [4 * (c % 4) + i for i in range(4)]
        rows = []
        for h in heads:
            rows.append(w[h * HD + PERM] * qscale)
        for h in heads:
            rows.append(w[C + h * HD + PERM])
        for h in heads:
            rows.append(w[2 * C + h * HD: 2 * C + (h + 1) * HD])
        wsel = np.concatenate(rows, axis=0)       # (768, C)
        xt = np.ascontiguousarray(x[b].T).reshape(NCT, 128, T).astype(BF16_NP)
        wt = np.ascontiguousarray(wsel.T).reshape(NCT, 128, 768).astype(BF16_NP)
        in_maps.append({"xt": xt, "wt": wt, "trig": trig})
    return in_maps


def kernel(x, w_attn, freqs_cos, freqs_sin):
    global LAST_RESULTS
    import os

    # The axon trace path needs antenv.axon_hooks, absent in this container.
    os.environ.pop("BASS_TRACE", None)
    from concourse.bass_utils import run_bass_kernel_spmd

    nc = _build_nc()
    in_maps = _host_prep(x, w_attn, freqs_cos, freqs_sin)
    res = run_bass_kernel_spmd(nc, in_maps, list(range(NCORES)))
    LAST_RESULTS = res
    y_full = np.zeros((B, NH, T, HD), np.float32)
    for c in range(NCORES):
        b = c // 4
        for i in range(4):
            y_full[b, 4 * (c % 4) + i] = res.results[c]["y"][i]
    return y_full


def bench(x, w_attn, freqs_cos, freqs_sin, iters=20):
    """Steady-state timing: device-resident inputs, repeated jitted execs.

    Returns (y_full, per_iter_seconds_min, per_iter_seconds_all)."""
    import time
    import jax
    from jax.sharding import Mesh, PartitionSpec
    from jax.experimental.shard_map import shard_map
    import concourse.mybir as mybir
    from concourse import bass2jax
    from concourse.bass2jax import _bass_exec_p, install_neuronx_cc_hook

    nc = _build_nc()
    install_neuronx_cc_hook()
    in_maps = _host_prep(x, w_attn, freqs_cos, freqs_sin)

    partition_name = nc.partition_id_tensor.name if nc.partition_id_tensor else None
    in_names, out_names, out_avals = [], [], []
    for alloc in nc.m.functions[0].allocations:
        if not isinstance(alloc, mybir.MemoryLocationSet):
            continue
        name = alloc.memorylocations[0].name
        if alloc.kind == "ExternalInput":
            if name != partition_name:
                in_names.append(name)
        elif alloc.kind == "ExternalOutput":
            out_names.append(name)
            out_avals.append(
                jax.core.ShapedArray(
                    tuple(alloc.tensor_shape), mybir.dt.np(alloc.dtype)
                )
            )

    n_params = len(in_names)
    all_names = in_names + out_names
    if partition_name is not None:
        all_names = all_names + [partition_name]

    def _body(*args):
        operands = list(args)
        if partition_name is not None:
            operands.append(bass2jax.partition_id_tensor())
        outs = _bass_exec_p.bind(
            *operands,
            out_avals=tuple(out_avals),
            in_names=tuple(all_names),
            out_names=tuple(out_names),
            lowering_input_output_aliases=(),
            sim_require_finite=False,
            sim_require_nnan=False,
            nc=nc,
        )
        return tuple(outs)

    devices = jax.devices()[:NCORES]
    mesh = Mesh(np.asarray(devices), ("core",))
    nouts = len(out_names)
    sharded = jax.jit(
        shard_map(
            _body,
            mesh=mesh,
            in_specs=(PartitionSpec("core"),) * (n_params + nouts),
            out_specs=(PartitionSpec("core"),) * nouts,
            check_rep=False,
        ),
        keep_unused=True,
    )
    concat_in = [
        np.concatenate([np.asarray(in_maps[c][nm]) for c in range(NCORES)], axis=0)
        for nm in in_names
    ]
    concat_zeros = [
        np.zeros((NCORES * a.shape[0], *a.shape[1:]), a.dtype) for a in out_avals
    ]
    args = [jax.device_put(a) for a in concat_in + concat_zeros]
    out = sharded(*args)
    jax.block_until_ready(out)
    times = []
    for _ in range(iters):
        t0 = time.perf_counter()
        out = sharded(*args)
        jax.block_until_ready(out)
        times.append(time.perf_counter() - t0)
    y_all = np.asarray(out[out_names.index("y")]).reshape(NCORES, 4, T, HD)
    y_full = np.zeros((B, NH, T, HD), np.float32)
    for c in range(NCORES):
        for i in range(4):
            y_full[c // 4, 4 * (c % 4) + i] = y_all[c, i]
    return y_full, min(times), times


# revision 64
# speedup vs baseline: 1.0169x; 1.0169x over previous
# Trainium2 Bass kernel for CausalSelfAttention (B=2, T=2048, C=1024, NH=16)
# with interleaved RoPE, sharded over 8 NeuronCores: each core computes one
# batch's 4 heads (data-parallel on batch x tensor-parallel on heads).
#
# Matmul datapath in bf16 (PE streams bf16 at 1 col/cycle @2.4GHz vs fp32's
# ~2.4 cycles/col); PSUM accumulation stays fp32.
#
# Per-core device algorithm:
#   inputs (host pre-laid-out): xt = x[b].T (C,T) bf16;  wt = Wsel.T (C,768)
#   bf16 where Wsel rows = [q-heads | k-heads | v-heads], q/k head rows
#   permuted to [e0..e15, o0..o15, e16..e31, o16..o31] so the RoPE partner
#   lives 16 partitions away inside a 32-partition quadrant; trig = (2,64,T)
#   bf16 RoPE multiplier patterns [CC, SS] (q versions pre-scaled by 1/8 via
#   the weights).
#   phase 1: qkT m-blocks (128 rows = 2 heads) = wt_m.T @ xt, RoPE applied as
#            qk' = raw*CC + shuffle16(raw)*SS  (DVE stream_shuffle swaps
#            16-row halves per quadrant; the two multiplies and the add run
#            on the otherwise-idle Pool engine); v = xt.T @ wt_v in natural
#            (T, d) layout with a ones-column appended (row-sum trick).
#   phase 2: per (head, 512-wide q-chunk): scoresT tiles (128 k, 512 q) on PE
#            into a 3-deep psum pool; causal masking of diagonal tiles via
#            gpsimd affine_select with fill=-1e30 ON THE PSUM, then a single
#            fused exp per 2-ktile group on ACT (no max-subtraction needed:
#            |scores| < ~4) writing bf16 probs; pv on PE accumulating
#            yT_ext (65, 512) whose row 64 = softmax denominators,
#            PE-transpose back to (q, d), scale by reciprocal, DMA out.
import sys

if "/opt/trn_rl_repo" not in sys.path:
    sys.path.insert(0, "/opt/trn_rl_repo")

import numpy as np
import ml_dtypes

BF16_NP = np.dtype(ml_dtypes.bfloat16)

B, T, C, NH, HD = 2, 2048, 1024, 16, 64
NCORES = 8
NCT = 8        # C tiles of 128
NCH = 4        # T chunks of 512
TCH = 512
NKT = 16       # k tiles of 128

PERM = np.array(
    [2 * i for i in range(16)]
    + [2 * i + 1 for i in range(16)]
    + [2 * i for i in range(16, 32)]
    + [2 * i + 1 for i in range(16, 32)],
    dtype=np.int64,
)
FREQ_OF_ROW = np.array(
    list(range(16)) + list(range(16)) + list(range(16, 32)) + list(range(16, 32)),
    dtype=np.int64,
)
IS_ODD_SLOT = np.array([0] * 16 + [1] * 16 + [0] * 16 + [1] * 16, dtype=np.int64)
SHUF_MASK = list(range(16, 32)) + list(range(16))

_CACHE: dict = {}
LAST_RESULTS = None


def _build_nc(phase2=True, rope=True, do_exp=True, do_mask=True, sps_bufs=2,
              pvps_bufs=2, stage_bufs=2, probs_bufs=4, key=None):
    ck = key or (phase2, rope, do_exp, do_mask, sps_bufs, pvps_bufs, stage_bufs,
                 probs_bufs)
    if ck in _CACHE:
        return _CACHE[ck]
    from concourse import bacc
    import concourse.tile as tile
    import concourse.mybir as mybir
    from concourse.masks import make_identity

    F32 = mybir.dt.float32
    BF16 = mybir.dt.bfloat16
    Exp = mybir.ActivationFunctionType.Exp
    Copy = mybir.ActivationFunctionType.Copy

    nc = bacc.Bacc(
        "TRN2",
        target_bir_lowering=False,
        debug=False,
        enable_asserts=False,
        num_devices=NCORES,
    )
    xt_d = nc.dram_tensor("xt", [NCT, 128, T], BF16, kind="ExternalInput")
    wt_d = nc.dram_tensor("wt", [NCT, 128, 768], BF16, kind="ExternalInput")
    trig_d = nc.dram_tensor("trig", [2, 64, T], BF16, kind="ExternalInput")
    y_d = nc.dram_tensor("y", [4, T, HD], F32, kind="ExternalOutput")

    with tile.TileContext(nc) as tc:
        with (
            tc.tile_pool(name="const", bufs=1) as constp,
            tc.tile_pool(name="p2sb", bufs=2) as p2sb,
            tc.tile_pool(name="rope", bufs=2) as ropep,
            tc.tile_pool(name="xw", bufs=1) as xwp,
            tc.tile_pool(name="stage", bufs=stage_bufs, space="PSUM") as stagep,
            tc.tile_pool(name="sps", bufs=sps_bufs, space="PSUM") as sps,
            tc.tile_pool(name="pvps", bufs=pvps_bufs, space="PSUM") as pvps,
        ):
            # ---- constants / destination tiles ----
            trig_t = [constp.tile([128, T], BF16, tag=f"trig{i}", name=f"trig{i}")
                      for i in range(2)]
            ident = constp.tile([128, 128], BF16, tag="ident", name="ident")
            make_identity(nc, ident[:])
            qkT = [
                [constp.tile([128, TCH], BF16, tag=f"qk{m}_{j}", name=f"qk{m}_{j}")
                 for j in range(NCH)]
                for m in range(4)
            ]
            vt = [constp.tile([128, 4, 65], BF16, tag=f"v{kt}", name=f"v{kt}")
                  for kt in range(NKT)]
            for kt in range(NKT):
                nc.vector.memset(vt[kt][:, :, 64:65], 1.0)

            # ---- input DMAs: w + x chunk 0 first, trig, then x chunks 1-3
            # (all on the sync queue: splitting across engine queues was
            # measured 3.5µs SLOWER on every core)
            x_t, w_t = [], []
            for ct in range(NCT):
                wtile = xwp.tile([128, 768], BF16, tag=f"w{ct}", name=f"w{ct}")
                nc.sync.dma_start(out=wtile, in_=wt_d[ct])
                w_t.append(wtile)
                xtile = xwp.tile([128, NCH, TCH], BF16, tag=f"x{ct}", name=f"x{ct}")
                nc.sync.dma_start(out=xtile[:, 0, :], in_=xt_d[ct, :, 0:TCH])
                x_t.append(xtile)
            for i in range(2):
                nc.sync.dma_start(out=trig_t[i][0:64, :], in_=trig_d[i])
                nc.sync.dma_start(out=trig_t[i][64:128, :], in_=trig_d[i])
            for j in range(1, NCH):
                for ct in range(NCT):
                    nc.sync.dma_start(
                        out=x_t[ct][:, j, :], in_=xt_d[ct, :, TCH * j: TCH * (j + 1)]
                    )

            # ---- per-group matmul + drain helpers ----
            def qk_mm(ps, m, j, u):
                nc.tensor.matmul(
                    ps,
                    w_t[u][:, 128 * m: 128 * (m + 1)],
                    x_t[u][:, j, :],
                    start=(u == 0),
                    stop=(u == NCT - 1),
                )

            def qk_drain(ps, m, j):
                if rope:
                    # ACT drains psum->bf16 (it has slack; psum reads are
                    # slow on DVE), then the whole RoPE chain runs on DVE in
                    # pure bf16 (2x 16-bit mode, SBUF-only operands)
                    raw = ropep.tile([128, TCH], BF16, tag="raw", name="raw")
                    nc.scalar.activation(raw, ps, Copy)
                    shuf = ropep.tile([128, TCH], BF16, tag="shuf", name="shuf")
                    nc.vector.stream_shuffle(out=shuf, in_=raw, mask=SHUF_MASK)
                    t1 = ropep.tile([128, TCH], BF16, tag="t1", name="t1")
                    nc.vector.tensor_mul(
                        t1, raw, trig_t[0][:, TCH * j: TCH * (j + 1)]
                    )
                    nc.vector.tensor_mul(
                        shuf, shuf, trig_t[1][:, TCH * j: TCH * (j + 1)]
                    )
                    nc.vector.tensor_add(qkT[m][j], t1, shuf)
                else:
                    nc.vector.tensor_copy(out=qkT[m][j], in_=ps)

            def v_mm(ps, kt, u):
                nc.tensor.matmul(
                    ps,
                    x_t[u][:, kt // 4, 128 * (kt % 4): 128 * (kt % 4) + 128],
                    w_t[u][:, 512:768],
                    start=(u == 0),
                    stop=(u == NCT - 1),
                )

            def v_drain(ps, kt):
                nc.vector.tensor_copy(
                    out=vt[kt][:, :, 0:64],
                    in_=ps.rearrange("p (s d) -> p s d", s=4),
                )

            def stage_wave_quanta(groups):
                # chop each ping-pong pair into 8 u-step quanta (2 matmuls
                # each) + 1 drain quantum, so the scheduler can interleave
                # them into the PE's exp-wait slots inside attention chunks
                quanta = []
                for i in range(0, len(groups), 2):
                    pair = groups[i: i + 2]
                    state = {}

                    def alloc(pair=pair, state=state):
                        if "pss" not in state:
                            state["pss"] = [
                                stagep.tile(
                                    [128, TCH] if kind == "qk" else [128, 256],
                                    F32, tag="st", name="stg",
                                )
                                for kind, a, b in pair
                            ]
                        return state["pss"]

                    for u in range(NCT):
                        def qmm(u=u, pair=pair, alloc=alloc):
                            for (kind, a, b), ps in zip(pair, alloc()):
                                if kind == "qk":
                                    qk_mm(ps, a, b, u)
                                else:
                                    v_mm(ps, a, u)
                        quanta.append(qmm)

                    def qdr(pair=pair, alloc=alloc):
                        for (kind, a, b), ps in zip(pair, alloc()):
                            if kind == "qk":
                                qk_drain(ps, a, b)
                            else:
                                v_drain(ps, a)
                    quanta.append(qdr)
                return quanta

            def stage_waves(groups):
                for q in stage_wave_quanta(groups):
                    q()

            filler = []

            def emit_filler(n):
                for _ in range(n):
                    if filler:
                        filler.pop(0)()

            # ---- attention machinery ----
            pending = [None]

            def finalize(h, j, psum_y):
                yt_sb = p2sb.tile([65, TCH], BF16, tag="yt", name="yt_sb")
                nc.vector.tensor_copy(out=yt_sb, in_=psum_y)
                psum_t = pvps.tile([128, 4, 66], BF16, tag="pv", name="ps_t")
                for s in range(4):
                    nc.tensor.transpose(
                        psum_t[:, s, 0:65],
                        yt_sb[:, 128 * s: 128 * (s + 1)],
                        ident[0:65, 0:65],
                    )
                rec = p2sb.tile([128, 4], F32, tag="rec", name="rec")
                nc.vector.reciprocal(out=rec, in_=psum_t[:, :, 64])
                y_sb = p2sb.tile([128, 4, HD], F32, tag="ysb", name="y_sb")
                for s in range(4):
                    nc.vector.tensor_scalar_mul(
                        out=y_sb[:, s, :],
                        in0=psum_t[:, s, 0:HD],
                        scalar1=rec[:, s: s + 1],
                    )
                nc.sync.dma_start(
                    out=y_d[h, TCH * j: TCH * (j + 1), :].rearrange(
                        "(s p) d -> p s d", p=128
                    ),
                    in_=y_sb,
                )

            pre_emitted = {}

            def make_emitter(h, j):
                qrow = 64 * (h % 2)
                qm, km = h // 2, 2 + h // 2
                qslice = qkT[qm][j][qrow: qrow + 64, :]
                stiles = {}

                def emit_scores(g):
                    ps = sps.tile([128, 2, TCH], F32, tag="s", name="ps_s")
                    for u in range(2):
                        ki = 2 * g + u
                        delta = max(0, 128 * (ki - 4 * j))
                        kslice = qkT[km][ki // 4][
                            qrow: qrow + 64,
                            128 * (ki % 4): 128 * (ki % 4 + 1),
                        ]
                        nc.tensor.matmul(
                            ps[:, u, delta:TCH], kslice, qslice[:, delta:TCH]
                        )
                    stiles[g] = ps

                return emit_scores, stiles

            def attn_chunk(h, j, nxt=None):
                nk = 4 * j + 4
                ng = nk // 2
                if (h, j) in pre_emitted:
                    emit_scores, stiles = pre_emitted.pop((h, j))
                else:
                    emit_scores, stiles = make_emitter(h, j)
                    emit_scores(0)
                if ng > 1:
                    emit_scores(1)
                if pending[0] is not None:
                    pending[0]()
                    pending[0] = None
                psum_y = pvps.tile([65, TCH], F32, tag="pv", name="pv")
                for g in range(ng):
                    psum_s = stiles.pop(g)
                    deltas = [128 * (2 * g + u - 4 * j) for u in range(2)]
                    dmin = max(0, deltas[0])
                    probs = p2sb.tile(
                        [128, 2, TCH], BF16, tag="probs", name="probs",
                        bufs=probs_bufs,
                    )
                    nc.scalar.activation(
                        probs[:, :, dmin:TCH],
                        psum_s[:, :, dmin:TCH],
                        Exp if do_exp else Copy,
                    )
                    if do_mask and deltas[1] > -128:
                        # causal mask of the diagonal 128-blocks (upper
                        # triangle of block -> 0), on the bf16 probs
                        for u in range(2):
                            d = max(0, deltas[u])
                            nc.gpsimd.affine_select(
                                out=probs[:, u, d:d + 128],
                                in_=probs[:, u, d:d + 128],
                                pattern=[[1, 128]],
                                compare_op=mybir.AluOpType.is_ge,
                                fill=0.0,
                                base=0,
                                channel_multiplier=-1,
                            )
                    if g + 2 < ng:
                        emit_scores(g + 2)
                    elif g == ng - 1 and nxt is not None:
                        nem, nst = make_emitter(*nxt)
                        nem(0)
                        pre_emitted[nxt] = (nem, nst)
                    for u in range(2):
                        ki = 2 * g + u
                        d = max(0, 128 * (ki - 4 * j))
                        nc.tensor.matmul(
                            psum_y[:, d:TCH],
                            vt[ki][:, h, :],
                            probs[:, u, d:TCH],
                            start=(ki == 0),
                            stop=(ki == nk - 1),
                        )

                def fin(h=h, j=j, psum_y=psum_y):
                    finalize(h, j, psum_y)

                pending[0] = fin

            # ---- chunk-streamed schedule ----
            # stage j+1's phase-1 waves are chopped into quanta and fed into
            # the exp-wait slot of every group of stage j's attention chunks
            # (emit_filler in attn_chunk), so the PE always has independent
            # work while ACT runs; leftovers flush at the stage boundary.
            # Pre-emit crosses the stage boundary (the needed qkT drains
            # land many quanta ahead).
            chunk_seq = [(h, j) for j in range(NCH) for h in range(4)]

            def stage_groups(j):
                # qk/v mixed per pair: spreads the heavy RoPE drains (DVE)
                # across all four pairs instead of bunching them up front
                qks = [("qk", 0, j), ("qk", 2, j), ("qk", 1, j), ("qk", 3, j)]
                vs = [("v", kt, 0) for kt in range(4 * j, 4 * j + 4)]
                out = []
                for a, b in zip(qks, vs):
                    out += [a, b]
                return out

            stage_waves(stage_groups(0))
            if phase2:
                for j in range(NCH):
                    nxt_groups = stage_groups(j + 1) if j + 1 < NCH else None
                    for h in range(4):
                        pos = chunk_seq.index((h, j))
                        nxt = chunk_seq[pos + 1] if pos + 1 < len(chunk_seq) else None
                        attn_chunk(h, j, nxt=nxt)
                        if nxt_groups is not None:
                            stage_waves(nxt_groups[2 * h: 2 * h + 2])
            else:
                for j in range(1, NCH):
                    stage_waves(stage_groups(j))
            if phase2 and pending[0] is not None:
                pending[0]()
                pending[0] = None

    nc.compile()
    _CACHE[ck] = nc
    return nc


def _host_prep(x, w_attn, freqs_cos, freqs_sin):
    x = np.asarray(x, dtype=np.float32)
    w = np.asarray(w_attn, dtype=np.float32)
    fc = np.asarray(freqs_cos, dtype=np.float32)
    fs = np.asarray(freqs_sin, dtype=np.float32)

    cosT, sinT = fc.T, fs.T                      # (32, T)
    CCp = cosT[FREQ_OF_ROW]                       # (64, T)
    SSp = sinT[FREQ_OF_ROW] * np.where(IS_ODD_SLOT == 1, 1.0, -1.0)[:, None].astype(
        np.float32
    )
    trig = np.ascontiguousarray(np.stack([CCp, SSp])).astype(BF16_NP)
    qscale = np.float32(1.0 / np.sqrt(HD))

    in_maps = []
    for c in range(NCORES):
        b = c // 4
        heads = [4 * (c % 4) + i for i in range(4)]
        rows = []
        for h in heads:
            rows.append(w[h * HD + PERM] * qscale)
        for h in heads:
            rows.append(w[C + h * HD + PERM])
        for h in heads:
            rows.append(w[2 * C + h * HD: 2 * C + (h + 1) * HD])
        wsel = np.concatenate(rows, axis=0)       # (768, C)
        xt = np.ascontiguousarray(x[b].T).reshape(NCT, 128, T).astype(BF16_NP)
        wt = np.ascontiguousarray(wsel.T).reshape(NCT, 128, 768).astype(BF16_NP)
        in_maps.append({"xt": xt, "wt": wt, "trig": trig})
    return in_maps


def kernel(x, w_attn, freqs_cos, freqs_sin):
    global LAST_RESULTS
    import os

    # The axon trace path needs antenv.axon_hooks, absent in this container.
    os.environ.pop("BASS_TRACE", None)
    from concourse.bass_utils import run_bass_kernel_spmd

    nc = _build_nc()
    in_maps = _host_prep(x, w_attn, freqs_cos, freqs_sin)
    res = run_bass_kernel_spmd(nc, in_maps, list(range(NCORES)))
    LAST_RESULTS = res
    y_full = np.zeros((B, NH, T, HD), np.float32)
    for c in range(NCORES):
        b = c // 4
        for i in range(4):
            y_full[b, 4 * (c % 4) + i] = res.results[c]["y"][i]
    return y_full


def bench(x, w_attn, freqs_cos, freqs_sin, iters=20):
    """Steady-state timing: device-resident inputs, repeated jitted execs.

    Returns (y_full, per_iter_seconds_min, per_iter_seconds_all)."""
    import time
    import jax
    from jax.sharding import Mesh, PartitionSpec
    from jax.experimental.shard_map import shard_map
    import concourse.mybir as mybir
    from concourse import bass2jax
    from concourse.bass2jax import _bass_exec_p, install_neuronx_cc_hook

    nc = _build_nc()
    install_neuronx_cc_hook()
    in_maps = _host_prep(x, w_attn, freqs_cos, freqs_sin)

    partition_name = nc.partition_id_tensor.name if nc.partition_id_tensor else None
    in_names, out_names, out_avals = [], [], []
    for alloc in nc.m.functions[0].allocations:
        if not isinstance(alloc, mybir.MemoryLocationSet):
            continue
        name = alloc.memorylocations[0].name
        if alloc.kind == "ExternalInput":
            if name != partition_name:
                in_names.append(name)
        elif alloc.kind == "ExternalOutput":
            out_names.append(name)
            out_avals.append(
                jax.core.ShapedArray(
                    tuple(alloc.tensor_shape), mybir.dt.np(alloc.dtype)
                )
            )

    n_params = len(in_names)
    all_names = in_names + out_names
    if partition_name is not None:
        all_names = all_names + [partition_name]

    def _body(*args):
        operands = list(args)
        if partition_name is not None:
            operands.append(bass2jax.partition_id_tensor())
        outs = _bass_exec_p.bind(
            *operands,
            out_avals=tuple(out_avals),
            in_names=tuple(all_names),
            out_names=tuple(out_names),
            lowering_input_output_aliases=(),
            sim_require_finite=False,
            sim_require_nnan=False,
            nc=nc,
        )
        return tuple(outs)

    devices = jax.devices()[:NCORES]
    mesh = Mesh(np.asarray(devices), ("core",))
    nouts = len(out_names)
    sharded = jax.jit(
        shard_map(
            _body,
            mesh=mesh,
            in_specs=(PartitionSpec("core"),) * (n_params + nouts),
            out_specs=(PartitionSpec("core"),) * nouts,
            check_rep=False,
        ),
        keep_unused=True,
    )
    concat_in = [
        np.concatenate([np.asarray(in_maps[c][nm]) for c in range(NCORES)], axis=0)
        for nm in in_names
    ]
    concat_zeros = [
        np.zeros((NCORES * a.shape[0], *a.shape[1:]), a.dtype) for a in out_avals
    ]
    args = [jax.device_put(a) for a in concat_in + concat_zeros]
    out = sharded(*args)
    jax.block_until_ready(out)
    times = []
    for _ in range(iters):
        t0 = time.perf_counter()
        out = sharded(*args)
        jax.block_until_ready(out)
        times.append(time.perf_counter() - t0)
    y_all = np.asarray(out[out_names.index("y")]).reshape(NCORES, 4, T, HD)
    y_full = np.zeros((B, NH, T, HD), np.float32)
    for c in range(NCORES):
        for i in range(4):
            y_full[c // 4, 4 * (c % 4) + i] = y_all[c, i]
    return y_full, min(times), times
